# revision 43
# baseline (speedup 1.0000x reference)
"""Causal multi-head attention (B=2, S=2048, D=2048, H=16) on 8 TRN2 cores.

Sharding: core c = (batch b = c//4, head-group r = c%4 -> heads 4r..4r+3).
Per core: project q/k/v for its 4 heads over all tokens, RoPE, exact-causal
attention in transposed-score layout (scoresT[keys, q] via lhsT=k_fm,
rhs=q_fm; z[dv, q] via lhsT=v_tokmajor, rhs=expT), output-projection
partials, per-phase fp16 ReduceScatter across the 4 cores of each batch.

Numerics: fp16 matmul inputs everywhere with fp32 PSUM accumulation; the
1/sqrt(dh) score scale is folded into the q-side RoPE tables; exp is biased
by -2 so fp16 exp sums stay in range.  Measured end-to-end rel err ~9e-4
(gate 2e-2).

Perf structure: all four weight matrices stay resident in SBUF (loaded
once), phases of 512 tokens pipeline proj(T+1) against attn(T)/wo(T); the
causal diagonal runs at 256-query granularity (saves tensor-engine rows);
phase 3 runs query-sub-major so its output projection + ReduceScatter split
in two and the final collective only exposes ~20us of tail.
"""
import sys

sys.path.insert(0, "/opt/trn_rl_repo")

from contextlib import ExitStack

import numpy as np

import concourse.bass as bass  # noqa: F401  (bass must import before tile)
import concourse.mybir as mybir
import concourse.tile as tile
from concourse import bacc, bass_isa
from concourse.bass_utils import run_bass_kernel_spmd

dt = mybir.dt
P = 128
D = 2048
N_HEAD = 16
DH = 128
HPC = 4            # heads per core
ROPE_BASE = 10000.0
GROUPS = [[0, 1, 2, 3], [4, 5, 6, 7]]
EXP_SHIFT = -2.0   # exp(s + EXP_SHIFT): keeps fp16 denominators < 65504


def _build(S: int):
    NP = S // 512  # token phases
    f16, f32 = dt.float16, dt.float32
    Exp = mybir.ActivationFunctionType.Exp
    nc = bacc.Bacc(None, target_bir_lowering=False, num_devices=8)

    xT = nc.declare_dram_parameter("xT", [D, S], f16, isOutput=False)
    wqT = nc.declare_dram_parameter("wqT", [D, 512], f16, isOutput=False)
    wkT = nc.declare_dram_parameter("wkT", [D, 512], f16, isOutput=False)
    wvT = nc.declare_dram_parameter("wvT", [D, 512], f16, isOutput=False)
    woT = nc.declare_dram_parameter("woT", [512, D], f16, isOutput=False)
    cosq = nc.declare_dram_parameter("cosq", [P, S], f16, isOutput=False)
    sinq = nc.declare_dram_parameter("sinq", [P, S], f16, isOutput=False)
    cosk = nc.declare_dram_parameter("cosk", [P, S], f16, isOutput=False)
    sink = nc.declare_dram_parameter("sink", [P, S], f16, isOutput=False)
    masks = nc.declare_dram_parameter("masks", [P, 1024], f16, isOutput=False)
    out_sh = nc.declare_dram_parameter("out_sh", [NP, 512, 512], f16,
                                       isOutput=True)

    rs_in = [nc.dram_tensor(f"rs_in{T}", [D, 512], f16) for T in range(NP - 1)]
    rs_out = [nc.dram_tensor(f"rs_out{T}", [512, 512], f16)
              for T in range(NP - 1)]
    HW_SPLIT = (128, 384)  # last-phase query split for the tail RS pair
    rs_in_h = [nc.dram_tensor(f"rs_in_h{u}", [D, w], f16)
               for u, w in enumerate(HW_SPLIT)]
    rs_out_h = [nc.dram_tensor(f"rs_out_h{u}", [512, w], f16)
                for u, w in enumerate(HW_SPLIT)]

    xT_r = xT.rearrange("(kt p) s -> p kt s", p=P)  # noqa: E501
    wq_r = wqT.rearrange("(kt p) n -> p kt n", p=P)
    wk_r = wkT.rearrange("(kt p) n -> p kt n", p=P)
    wv_r = wvT.rearrange("(kt p) n -> p kt n", p=P)
    wo_r = woT.rearrange("(kt p) n -> p kt n", p=P)

    with tile.TileContext(nc) as tc, ExitStack() as ctx:
        const = ctx.enter_context(tc.tile_pool(name="const", bufs=1))
        wpool = ctx.enter_context(tc.tile_pool(name="wpool", bufs=1))
        kvres = ctx.enter_context(tc.tile_pool(name="kvres", bufs=1))
        xp = ctx.enter_context(tc.tile_pool(name="xp", bufs=2))
        qp = ctx.enter_context(tc.tile_pool(name="qp", bufs=2))
        zp = ctx.enter_context(tc.tile_pool(name="zp", bufs=2))
        rp = ctx.enter_context(tc.tile_pool(name="rp", bufs=2))
        ep = ctx.enter_context(tc.tile_pool(name="ep", bufs=9))
        dp = ctx.enter_context(tc.tile_pool(name="dp", bufs=5))
        emp = ctx.enter_context(tc.tile_pool(name="emp", bufs=4))
        bp = ctx.enter_context(tc.tile_pool(name="bp", bufs=2))
        op_ = ctx.enter_context(tc.tile_pool(name="op", bufs=2))
        pp = ctx.enter_context(tc.tile_pool(name="pp", bufs=2, space="PSUM"))
        ps_s = ctx.enter_context(tc.tile_pool(name="ps_s", bufs=2, space="PSUM"))
        ps_z = ctx.enter_context(tc.tile_pool(name="ps_z", bufs=4, space="PSUM"))

        # ---- resident weights + constants -------------------------------
        # Load order matters: the SP sequencer + HWDGE serialize DMA issue,
        # so interleave wq with x(0) (both gate the first matmul chain) and
        # defer wk/wv/wo/attn constants past them.
        wq_sb = wpool.tile([P, 16, 512], f16, tag="wq", name="wq_sb")
        wk_sb = wpool.tile([P, 16, 512], f16, tag="wk", name="wk_sb")
        wv_sb = wpool.tile([P, 16, 512], f16, tag="wv", name="wv_sb")
        wo_sb = wpool.tile([P, 4, 2048], f16, tag="wo", name="wo_sb")
        x_sb0 = xp.tile([P, 16, 512], f16, tag="x", name="x_sb0")
        cq_sb = const.tile([P, S], f16, tag="cq", name="cq_sb")
        sq_sb = const.tile([P, S], f16, tag="sq", name="sq_sb")
        ck_sb = const.tile([P, S], f16, tag="ck", name="ck_sb")
        sk_sb = const.tile([P, S], f16, tag="sk", name="sk_sb")
        masks_sb = const.tile([P, 1024], f16, tag="masks", name="masks_sb")
        ebias_sb = const.tile([P, 1], f32, tag="ebias", name="ebias_sb")
        nc.vector.memset(ebias_sb, EXP_SHIFT)
        # 2-kd first chunks so the first matmul chain starts ~3us in; rope
        # tables land right after the wq/x stream (the h2 projection chain
        # recycles h0's PSUM bank, whose readers need cq/sq)
        for a, b in ((0, 2), (2, 4)):
            nc.sync.dma_start(out=wq_sb[:, a:b, :], in_=wq_r[:, a:b, :])
            nc.sync.dma_start(out=x_sb0[:, a:b, :],
                              in_=xT_r[:, a:b, 0:512])
        for c in range(1, 4):
            nc.sync.dma_start(out=wq_sb[:, 4 * c:4 * c + 4, :],
                              in_=wq_r[:, 4 * c:4 * c + 4, :])
            nc.sync.dma_start(out=x_sb0[:, 4 * c:4 * c + 4, :],
                              in_=xT_r[:, 4 * c:4 * c + 4, 0:512])
        nc.sync.dma_start(out=cq_sb[:, 0:512], in_=cosq[:, 0:512])
        nc.sync.dma_start(out=sq_sb[:, 0:512], in_=sinq[:, 0:512])
        for c in range(4):
            nc.sync.dma_start(out=wk_sb[:, 4 * c:4 * c + 4, :],
                              in_=wk_r[:, 4 * c:4 * c + 4, :])
        nc.sync.dma_start(out=ck_sb[:, 0:512], in_=cosk[:, 0:512])
        nc.sync.dma_start(out=sk_sb[:, 0:512], in_=sink[:, 0:512])
        for c in range(4):
            nc.sync.dma_start(out=wv_sb[:, 4 * c:4 * c + 4, :],
                              in_=wv_r[:, 4 * c:4 * c + 4, :])
        nc.sync.dma_start(out=masks_sb, in_=masks[:, :])
        for c in range(4):
            nc.sync.dma_start(out=wo_sb[:, c, :], in_=wo_r[:, c, :])

        # persistent K (feature-major) and V (token-major) per phase
        k_sbs = [kvres.tile([P, HPC, 512], f16, tag=f"k{T}", name=f"k_sb{T}")
                 for T in range(NP)]
        v_sbs = [kvres.tile([P, 4, 512], f16, tag=f"v{T}", name=f"v_sb{T}")
                 for T in range(NP)]

        q_sbs = {}
        z_sbs = {}

        def proj_qk(T, x_pre=None):
            tok = slice(512 * T, 512 * (T + 1))
            if x_pre is None:
                x_sb = xp.tile([P, 16, 512], f16, tag="x", name=f"x_sb{T}")
                for c in range(4):
                    nc.sync.dma_start(out=x_sb[:, 4 * c:4 * c + 4, :],
                                      in_=xT_r[:, 4 * c:4 * c + 4, tok])
                nc.sync.dma_start(out=cq_sb[:, tok], in_=cosq[:, tok])
                nc.sync.dma_start(out=sq_sb[:, tok], in_=sinq[:, tok])
                nc.sync.dma_start(out=ck_sb[:, tok], in_=cosk[:, tok])
                nc.sync.dma_start(out=sk_sb[:, tok], in_=sink[:, tok])
            else:
                x_sb = x_pre

            q_sb = qp.tile([P, HPC, 512], f16, tag="q", name=f"q_sb{T}")
            q_sbs[T] = q_sb
            for w_sb, ct, st, is_q in ((wq_sb, cq_sb, sq_sb, True),
                                       (wk_sb, ck_sb, sk_sb, False)):
                for h in range(HPC):
                    ps = pp.tile([P, 512], f32, tag="pp",
                                 name=f"psqk{T}{int(is_q)}{h}")
                    for kd in range(16):
                        nc.tensor.matmul(ps[:],
                                         lhsT=w_sb[:, kd, P * h:P * (h + 1)],
                                         rhs=x_sb[:, kd, :],
                                         start=(kd == 0), stop=(kd == 15))
                    # rotate_half via two ACT copies (partition-shifted,
                    # negated upper half); keeps the tensor engine free
                    rot = rp.tile([P, 512], f16, tag="rot")
                    nc.scalar.activation(rot[0:64, :], ps[64:128, :],
                                         mybir.ActivationFunctionType.Copy,
                                         scale=-1.0)
                    nc.scalar.copy(rot[64:128, :], ps[0:64, :])
                    t1 = rp.tile([P, 512], f16, tag="t1")
                    nc.vector.tensor_mul(t1[:], ps[:], ct[:, tok])
                    swp = rp.tile([P, 512], f16, tag="swp")
                    nc.vector.tensor_mul(swp[:], rot[:], st[:, tok])
                    dst = q_sb[:, h, :] if is_q else k_sbs[T][:, h, :]
                    nc.vector.tensor_add(dst, t1[:], swp[:])

            return x_sb

        def proj_v(T, x_sb, chunks=range(4)):
            for i in chunks:
                ps = pp.tile([P, 512], f32, tag="pp", name=f"psv{T}{i}")
                for kd in range(16):
                    nc.tensor.matmul(ps[:],
                                     lhsT=x_sb[:, kd, P * i:P * (i + 1)],
                                     rhs=wv_sb[:, kd, :],
                                     start=(kd == 0), stop=(kd == 15))
                nc.scalar.copy(v_sbs[T][:, i, :], ps[:])

        def proj_phase(T, x_pre=None):
            proj_v(T, proj_qk(T, x_pre))

        def _chunk(kb, h, q_sb, ps_zt, den, qlo, qhi, mask_idx,
                   z_start, z_stop, den_first):
            """One 128-key score/exp/den/z step over queries [qlo, qhi)."""
            w = qhi - qlo
            ps = ps_s.tile([P, 512], f32, tag="s")
            nc.tensor.matmul(
                ps[:, :w],
                lhsT=k_sbs[kb // 4][:, h, P * (kb % 4):P * (kb % 4 + 1)],
                rhs=q_sb[:, h, qlo:qhi],
                start=True, stop=True, skip_group_check=True)
            et = ep.tile([P, 512], f16, tag="et")
            nc.scalar.activation(et[:, :w], ps[:, :w], Exp, bias=ebias_sb[:])
            if mask_idx is not None:
                c = 384 - 128 * mask_idx
                em = emp.tile([P, 512], f16, tag="em")
                nc.vector.tensor_mul(em[:, :w], et[:, :w],
                                     masks_sb[:, c:c + w])
                e_use = em
            else:
                e_use = et
            if den_first:
                nc.vector.tensor_copy(den[:, qlo:qhi], e_use[:, :w])
            else:
                nc.vector.tensor_add(den[:, qlo:qhi], den[:, qlo:qhi],
                                     e_use[:, :w])
            nc.tensor.matmul(
                ps_zt[:, qlo:qhi],
                lhsT=v_sbs[kb // 4][:, kb % 4, P * h:P * (h + 1)],
                rhs=e_use[:, :w],
                start=z_start, stop=z_stop, skip_group_check=True)

        def attn_phase(T):
            """Head-major attention for phases 0..NP-2: shared 512-wide
            rectangle + 256-wide diagonal sub-blocks."""
            q_sb = q_sbs.pop(T)
            z_sb = zp.tile([P, HPC, 512], f16, tag="z", name=f"z_sb{T}")
            for h in range(HPC):
                ps_zt = ps_z.tile([P, 512], f32, tag="z")
                den = dp.tile([P, 512], f16, tag="den")
                for kb in range(4 * T):  # full-width rectangle
                    _chunk(kb, h, q_sb, ps_zt, den, 0, 512, None,
                           z_start=(kb == 0), z_stop=False,
                           den_first=(kb == 0))
                for i in range(2):       # 256-wide diagonal
                    for j in range(2 * (i + 1)):
                        _chunk(4 * T + j, h, q_sb, ps_zt, den,
                               256 * i, 256 * (i + 1),
                               (j - 2 * i) if j >= 2 * i else None,
                               z_start=(T == 0 and j == 0),
                               z_stop=(j == 2 * i + 1),
                               den_first=(T == 0 and j == 0))
                ds = bp.tile([P, 512], f32, tag="ds")
                nc.gpsimd.partition_all_reduce(ds[:], den[:], channels=P,
                                               reduce_op=bass_isa.ReduceOp.add)
                bc = bp.tile([P, 512], f32, tag="bc")
                nc.vector.reciprocal(bc[:], ds[:])
                nc.vector.tensor_mul(z_sb[:, h, :], ps_zt[:], bc[:])
            z_sbs[T] = z_sb

        def attn3_rect(T):
            """Last phase, stage 1: full-width rectangle (keys < 512T) for
            all heads.  Emitted between proj_qk(T) and proj_v(T) so its
            exp load runs under the projection instead of in the tail."""
            q_sb = q_sbs[T]
            zts, dens = [], []
            for h in range(HPC):
                ps_zt = ps_z.tile([P, 512], f32, tag="z", name=f"z3r{h}")
                den = dp.tile([P, 512], f16, tag="den", name=f"den3{h}")
                for kb in range(4 * T):
                    _chunk(kb, h, q_sb, ps_zt, den, 0, 512, None,
                           z_start=(kb == 0), z_stop=False,
                           den_first=(kb == 0))
                zts.append(ps_zt)
                dens.append(den)
            return zts, dens

        def attn3_diag(T, lo, hi, chunks, zts, dens, z_sb):
            """Last phase, stage 2: diagonal chunks for queries [lo, hi),
            then normalize that slice of z (feeds wo_last_part)."""
            q_sb = q_sbs[T]
            for h in range(HPC):
                for n, (kb, mi) in enumerate(chunks):
                    _chunk(kb, h, q_sb, zts[h], dens[h], lo, hi, mi,
                           z_start=False, z_stop=(n == len(chunks) - 1),
                           den_first=False)
                ds = bp.tile([P, 512], f32, tag="ds")
                nc.gpsimd.partition_all_reduce(
                    ds[:, lo:hi], dens[h][:, lo:hi], channels=P,
                    reduce_op=bass_isa.ReduceOp.add)
                bc = bp.tile([P, 512], f32, tag="bc")
                nc.vector.reciprocal(bc[:, lo:hi], ds[:, lo:hi])
                nc.vector.tensor_mul(z_sb[:, h, lo:hi],
                                     zts[h][:, lo:hi], bc[:, lo:hi])

        def wo_phase(T):
            z_sb = z_sbs.pop(T)
            rs_r = rs_in[T].rearrange("(g mi p) n -> p g mi n", p=P, mi=4)
            for g in range(4):
                o4 = op_.tile([P, 4, 512], f16, tag="o_t")
                for mi in range(4):
                    m = 4 * g + mi
                    ps = ps_z.tile([P, 512], f32, tag="z", name=f"pso{T}{m}")
                    for kd in range(HPC):
                        nc.tensor.matmul(ps[:],
                                         lhsT=wo_sb[:, kd, P * m:P * (m + 1)],
                                         rhs=z_sb[:, kd, :],
                                         start=(kd == 0), stop=(kd == HPC - 1))
                    if m % 2 == 0:
                        nc.scalar.copy(o4[:, mi, :], ps[:])
                    else:
                        nc.vector.tensor_copy(o4[:, mi, :], ps[:])
                nc.gpsimd.dma_start(out=rs_r[:, g, :, :], in_=o4[:])
            nc.gpsimd.collective_compute(
                "ReduceScatter", mybir.AluOpType.add, replica_groups=GROUPS,
                ins=[rs_in[T][:, :]], outs=[rs_out[T][:, :]])
            nc.sync.dma_start(out=out_sh[T, :, :], in_=rs_out[T][:, :])

        def wo_last_part(u, lo, hi, z_sb):
            w = hi - lo
            # rs writes go out on an idle HWDGE queue (SP for the first
            # part, ACT for the second) -- Pool's SWDGE path serializes
            # ~1.1us per descriptor-gen right before the tail collectives
            dma_eng = nc.sync if u == 0 else nc.scalar
            rs_r = rs_in_h[u].rearrange("(g mi p) n -> p g mi n", p=P, mi=4)
            for g in range(4):
                o4 = op_.tile([P, 4, 512], f16, tag="o_t")
                for mi in range(4):
                    m = 4 * g + mi
                    ps = ps_z.tile([P, 512], f32, tag="z", name=f"psoh{u}{m}")
                    for kd in range(HPC):
                        nc.tensor.matmul(
                            ps[:, :w],
                            lhsT=wo_sb[:, kd, P * m:P * (m + 1)],
                            rhs=z_sb[:, kd, lo:hi],
                            start=(kd == 0), stop=(kd == HPC - 1),
                            skip_group_check=True)
                    nc.vector.tensor_copy(o4[:, mi, 0:w], ps[:, :w])
                dma_eng.dma_start(out=rs_r[:, g, :, :], in_=o4[:, :, 0:w])
            nc.gpsimd.collective_compute(
                "ReduceScatter", mybir.AluOpType.add, replica_groups=GROUPS,
                ins=[rs_in_h[u][:, :]], outs=[rs_out_h[u][:, :]])
            dma_eng.dma_start(out=out_sh[NP - 1, :, lo:hi],
                              in_=rs_out_h[u][:, :])

        TL = NP - 1
        for T in range(TL):
            if T >= 1:
                attn_phase(T - 1)
                wo_phase(T - 1)
            proj_phase(T, x_pre=x_sb0 if T == 0 else None)
        attn_phase(TL - 1)
        wo_phase(TL - 1)
        x3 = proj_qk(TL)
        z_last = zp.tile([P, HPC, 512], f16, tag="z", name="z_last")
        zts, dens = attn3_rect(TL)
        proj_v(TL, x3, chunks=(0,))
        attn3_diag(TL, 0, 128, [(4 * TL, 0)], zts, dens, z_last)
        wo_last_part(0, 0, 128, z_last)
        proj_v(TL, x3, chunks=(1, 2, 3))
        attn3_diag(TL, 128, 512,
                   [(4 * TL, None), (4 * TL + 1, 0),
                    (4 * TL + 2, 1), (4 * TL + 3, 2)],
                   zts, dens, z_last)
        wo_last_part(1, 128, 512, z_last)
        q_sbs.pop(TL)

    nc.compile()
    return nc


_BUILT = {}


def _get_built(S):
    if S not in _BUILT:
        _BUILT[S] = _build(S)
    return _BUILT[S]


def host_inputs(x, w_qkv, w_o):
    """Build the 8 per-core input maps from full inputs."""
    B, S, D_ = x.shape
    scale = np.float32(DH) ** -0.5

    j = np.arange(0, DH, 2, dtype=np.float32) / DH
    inv_freq = (1.0 / (ROPE_BASE ** j)).astype(np.float32)
    t = np.arange(S, dtype=np.float32)
    freqs = np.outer(inv_freq, t)                            # [64, S]
    emb = np.concatenate([freqs, freqs], axis=0)             # [128, S]
    cos_t = np.cos(emb)
    sin_t = np.sin(emb)
    cosq_t = (cos_t * scale).astype(np.float16)
    sinq_t = (sin_t * scale).astype(np.float16)
    cosk_t = cos_t.astype(np.float16)
    sink_t = sin_t.astype(np.float16)

    # masks[k, u] = (u >= k + 384): slicing at [384+c : 384+c+w] yields the
    # causal mask (q >= k + c) for a 128-key chunk against w queries
    u_idx = np.arange(1024)[None, :]
    k_idx = np.arange(P)[:, None]
    masks_np = (u_idx >= k_idx + 384).astype(np.float16)     # [128, 1024]

    wqkvT = w_qkv.T.astype(np.float16)       # [D, 3D]
    woT_full = w_o.T.astype(np.float16)      # [D(in), D(out)]
    xTb = [np.ascontiguousarray(x[b].T).astype(np.float16) for b in range(2)]

    in_maps = []
    for c in range(8):
        b, r = c // 4, c % 4
        in_maps.append({
            "xT": xTb[b],
            "wqT": np.ascontiguousarray(wqkvT[:, 512 * r:512 * (r + 1)]),
            "wkT": np.ascontiguousarray(
                wqkvT[:, D + 512 * r:D + 512 * (r + 1)]),
            "wvT": np.ascontiguousarray(
                wqkvT[:, 2 * D + 512 * r:2 * D + 512 * (r + 1)]),
            "woT": np.ascontiguousarray(woT_full[512 * r:512 * (r + 1), :]),
            "cosq": cosq_t, "sinq": sinq_t,
            "cosk": cosk_t, "sink": sink_t,
            "masks": masks_np,
        })
    return in_maps


def assemble(results, B, S):
    NP = S // 512
    out = np.empty((B, S, D), dtype=np.float32)
    for c in range(8):
        b, r = c // 4, c % 4
        sh = results[c]["out_sh"]  # [NP, 512(dout), 512(tok)] fp16
        for T in range(NP):
            out[b, 512 * T:512 * (T + 1), 512 * r:512 * (r + 1)] = \
                sh[T].T.astype(np.float32)
    return out


def kernel(x, w_qkv, w_o, _trace=False):
    x = np.asarray(x, dtype=np.float32)
    w_qkv = np.asarray(w_qkv, dtype=np.float32)
    w_o = np.asarray(w_o, dtype=np.float32)
    B, S, _ = x.shape
    nc = _get_built(S)
    in_maps = host_inputs(x, w_qkv, w_o)

    def _run():
        try:
            return run_bass_kernel_spmd(nc, in_maps, list(range(8)),
                                        trace=_trace)
        except ModuleNotFoundError:
            return run_bass_kernel_spmd(nc, in_maps, list(range(8)))

    try:
        res = _run()
    except Exception:
        res = _run()  # transient runtime/readback errors: retry once
    out = assemble(res.results, B, S)
    if _trace:
        return out, res
    return out


# revision 44
# speedup vs baseline: 1.0074x; 1.0074x over previous
"""Causal multi-head attention (B=2, S=2048, D=2048, H=16) on 8 TRN2 cores.

Sharding: core c = (batch b = c//4, head-group r = c%4 -> heads 4r..4r+3).
Per core: project q/k/v for its 4 heads over all tokens, RoPE, exact-causal
attention in transposed-score layout (scoresT[keys, q] via lhsT=k_fm,
rhs=q_fm; z[dv, q] via lhsT=v_tokmajor, rhs=expT), output-projection
partials, per-phase fp16 ReduceScatter across the 4 cores of each batch.

Numerics: fp16 matmul inputs everywhere with fp32 PSUM accumulation; the
1/sqrt(dh) score scale is folded into the q-side RoPE tables; exp is biased
by -2 so fp16 exp sums stay in range.  Measured end-to-end rel err ~9e-4
(gate 2e-2).

Perf structure: all four weight matrices stay resident in SBUF (loaded
once), phases of 512 tokens pipeline proj(T+1) against attn(T)/wo(T); the
causal diagonal runs at 256-query granularity (saves tensor-engine rows);
phase 3 runs query-sub-major so its output projection + ReduceScatter split
in two and the final collective only exposes ~20us of tail.
"""
import sys

sys.path.insert(0, "/opt/trn_rl_repo")

from contextlib import ExitStack

import numpy as np

import concourse.bass as bass  # noqa: F401  (bass must import before tile)
import concourse.mybir as mybir
import concourse.tile as tile
from concourse import bacc, bass_isa
from concourse.bass_utils import run_bass_kernel_spmd

dt = mybir.dt
P = 128
D = 2048
N_HEAD = 16
DH = 128
HPC = 4            # heads per core
ROPE_BASE = 10000.0
GROUPS = [[0, 1, 2, 3], [4, 5, 6, 7]]
EXP_SHIFT = -2.0   # exp(s + EXP_SHIFT): keeps fp16 denominators < 65504


def _build(S: int):
    NP = S // 512  # token phases
    f16, f32 = dt.float16, dt.float32
    Exp = mybir.ActivationFunctionType.Exp
    nc = bacc.Bacc(None, target_bir_lowering=False, num_devices=8)

    xT = nc.declare_dram_parameter("xT", [D, S], f16, isOutput=False)
    wqT = nc.declare_dram_parameter("wqT", [D, 512], f16, isOutput=False)
    wkT = nc.declare_dram_parameter("wkT", [D, 512], f16, isOutput=False)
    wvT = nc.declare_dram_parameter("wvT", [D, 512], f16, isOutput=False)
    woT = nc.declare_dram_parameter("woT", [512, D], f16, isOutput=False)
    cosq = nc.declare_dram_parameter("cosq", [P, S], f16, isOutput=False)
    sinq = nc.declare_dram_parameter("sinq", [P, S], f16, isOutput=False)
    cosk = nc.declare_dram_parameter("cosk", [P, S], f16, isOutput=False)
    sink = nc.declare_dram_parameter("sink", [P, S], f16, isOutput=False)
    masks = nc.declare_dram_parameter("masks", [P, 1024], f16, isOutput=False)
    out_sh = nc.declare_dram_parameter("out_sh", [NP, 512, 512], f16,
                                       isOutput=True)

    rs_in = [nc.dram_tensor(f"rs_in{T}", [D, 512], f16) for T in range(NP - 1)]
    rs_out = [nc.dram_tensor(f"rs_out{T}", [512, 512], f16)
              for T in range(NP - 1)]
    HW_SPLIT = (128, 384)  # last-phase query split for the tail RS pair
    rs_in_h = [nc.dram_tensor(f"rs_in_h{u}", [D, w], f16)
               for u, w in enumerate(HW_SPLIT)]
    rs_out_h = [nc.dram_tensor(f"rs_out_h{u}", [512, w], f16)
                for u, w in enumerate(HW_SPLIT)]

    xT_r = xT.rearrange("(kt p) s -> p kt s", p=P)  # noqa: E501
    wq_r = wqT.rearrange("(kt p) n -> p kt n", p=P)
    wk_r = wkT.rearrange("(kt p) n -> p kt n", p=P)
    wv_r = wvT.rearrange("(kt p) n -> p kt n", p=P)
    wo_r = woT.rearrange("(kt p) n -> p kt n", p=P)

    with tile.TileContext(nc) as tc, ExitStack() as ctx:
        const = ctx.enter_context(tc.tile_pool(name="const", bufs=1))
        wpool = ctx.enter_context(tc.tile_pool(name="wpool", bufs=1))
        kvres = ctx.enter_context(tc.tile_pool(name="kvres", bufs=1))
        xp = ctx.enter_context(tc.tile_pool(name="xp", bufs=2))
        qp = ctx.enter_context(tc.tile_pool(name="qp", bufs=2))
        zp = ctx.enter_context(tc.tile_pool(name="zp", bufs=2))
        rp = ctx.enter_context(tc.tile_pool(name="rp", bufs=2))
        ep = ctx.enter_context(tc.tile_pool(name="ep", bufs=9))
        dp = ctx.enter_context(tc.tile_pool(name="dp", bufs=5))
        emp = ctx.enter_context(tc.tile_pool(name="emp", bufs=4))
        bp = ctx.enter_context(tc.tile_pool(name="bp", bufs=2))
        op_ = ctx.enter_context(tc.tile_pool(name="op", bufs=2))
        pp = ctx.enter_context(tc.tile_pool(name="pp", bufs=2, space="PSUM"))
        ps_s = ctx.enter_context(tc.tile_pool(name="ps_s", bufs=2, space="PSUM"))
        ps_z = ctx.enter_context(tc.tile_pool(name="ps_z", bufs=4, space="PSUM"))

        # ---- resident weights + constants -------------------------------
        # Load order matters: the SP sequencer + HWDGE serialize DMA issue,
        # so interleave wq with x(0) (both gate the first matmul chain) and
        # defer wk/wv/wo/attn constants past them.
        wq_sb = wpool.tile([P, 16, 512], f16, tag="wq", name="wq_sb")
        wk_sb = wpool.tile([P, 16, 512], f16, tag="wk", name="wk_sb")
        wv_sb = wpool.tile([P, 16, 512], f16, tag="wv", name="wv_sb")
        wo_sb = wpool.tile([P, 4, 2048], f16, tag="wo", name="wo_sb")
        x_sb0 = xp.tile([P, 16, 512], f16, tag="x", name="x_sb0")
        cq_sb = const.tile([P, S], f16, tag="cq", name="cq_sb")
        sq_sb = const.tile([P, S], f16, tag="sq", name="sq_sb")
        ck_sb = const.tile([P, S], f16, tag="ck", name="ck_sb")
        sk_sb = const.tile([P, S], f16, tag="sk", name="sk_sb")
        masks_sb = const.tile([P, 1024], f16, tag="masks", name="masks_sb")
        ebias_sb = const.tile([P, 1], f32, tag="ebias", name="ebias_sb")
        nc.vector.memset(ebias_sb, EXP_SHIFT)
        # 2-kd first chunks so the first matmul chain starts ~3us in; rope
        # tables land right after the wq/x stream (the h2 projection chain
        # recycles h0's PSUM bank, whose readers need cq/sq)
        for a, b in ((0, 2), (2, 4)):
            nc.sync.dma_start(out=wq_sb[:, a:b, :], in_=wq_r[:, a:b, :])
            nc.sync.dma_start(out=x_sb0[:, a:b, :],
                              in_=xT_r[:, a:b, 0:512])
        for c in range(1, 4):
            nc.sync.dma_start(out=wq_sb[:, 4 * c:4 * c + 4, :],
                              in_=wq_r[:, 4 * c:4 * c + 4, :])
            nc.sync.dma_start(out=x_sb0[:, 4 * c:4 * c + 4, :],
                              in_=xT_r[:, 4 * c:4 * c + 4, 0:512])
        nc.sync.dma_start(out=cq_sb[:, 0:512], in_=cosq[:, 0:512])
        nc.sync.dma_start(out=sq_sb[:, 0:512], in_=sinq[:, 0:512])
        for c in range(4):
            nc.sync.dma_start(out=wk_sb[:, 4 * c:4 * c + 4, :],
                              in_=wk_r[:, 4 * c:4 * c + 4, :])
        nc.sync.dma_start(out=ck_sb[:, 0:512], in_=cosk[:, 0:512])
        nc.sync.dma_start(out=sk_sb[:, 0:512], in_=sink[:, 0:512])
        for c in range(4):
            nc.sync.dma_start(out=wv_sb[:, 4 * c:4 * c + 4, :],
                              in_=wv_r[:, 4 * c:4 * c + 4, :])
        nc.sync.dma_start(out=masks_sb, in_=masks[:, :])
        for c in range(4):
            nc.sync.dma_start(out=wo_sb[:, c, :], in_=wo_r[:, c, :])

        # persistent K (feature-major) and V (token-major) per phase
        k_sbs = [kvres.tile([P, HPC, 512], f16, tag=f"k{T}", name=f"k_sb{T}")
                 for T in range(NP)]
        v_sbs = [kvres.tile([P, 4, 512], f16, tag=f"v{T}", name=f"v_sb{T}")
                 for T in range(NP)]

        q_sbs = {}
        z_sbs = {}

        def proj_qk(T, x_pre=None):
            tok = slice(512 * T, 512 * (T + 1))
            if x_pre is None:
                x_sb = xp.tile([P, 16, 512], f16, tag="x", name=f"x_sb{T}")
                for c in range(4):
                    nc.sync.dma_start(out=x_sb[:, 4 * c:4 * c + 4, :],
                                      in_=xT_r[:, 4 * c:4 * c + 4, tok])
                nc.sync.dma_start(out=cq_sb[:, tok], in_=cosq[:, tok])
                nc.sync.dma_start(out=sq_sb[:, tok], in_=sinq[:, tok])
                nc.sync.dma_start(out=ck_sb[:, tok], in_=cosk[:, tok])
                nc.sync.dma_start(out=sk_sb[:, tok], in_=sink[:, tok])
            else:
                x_sb = x_pre

            q_sb = qp.tile([P, HPC, 512], f16, tag="q", name=f"q_sb{T}")
            q_sbs[T] = q_sb
            for w_sb, ct, st, is_q in ((wq_sb, cq_sb, sq_sb, True),
                                       (wk_sb, ck_sb, sk_sb, False)):
                for h in range(HPC):
                    ps = pp.tile([P, 512], f32, tag="pp",
                                 name=f"psqk{T}{int(is_q)}{h}")
                    for kd in range(16):
                        nc.tensor.matmul(ps[:],
                                         lhsT=w_sb[:, kd, P * h:P * (h + 1)],
                                         rhs=x_sb[:, kd, :],
                                         start=(kd == 0), stop=(kd == 15))
                    # rotate_half via two ACT copies (partition-shifted,
                    # negated upper half); keeps the tensor engine free
                    rot = rp.tile([P, 512], f16, tag="rot")
                    nc.scalar.activation(rot[0:64, :], ps[64:128, :],
                                         mybir.ActivationFunctionType.Copy,
                                         scale=-1.0)
                    nc.scalar.copy(rot[64:128, :], ps[0:64, :])
                    t1 = rp.tile([P, 512], f16, tag="t1")
                    nc.vector.tensor_mul(t1[:], ps[:], ct[:, tok])
                    swp = rp.tile([P, 512], f16, tag="swp")
                    nc.vector.tensor_mul(swp[:], rot[:], st[:, tok])
                    dst = q_sb[:, h, :] if is_q else k_sbs[T][:, h, :]
                    nc.vector.tensor_add(dst, t1[:], swp[:])

            return x_sb

        def proj_v(T, x_sb, chunks=range(4)):
            for i in chunks:
                ps = pp.tile([P, 512], f32, tag="pp", name=f"psv{T}{i}")
                for kd in range(16):
                    nc.tensor.matmul(ps[:],
                                     lhsT=x_sb[:, kd, P * i:P * (i + 1)],
                                     rhs=wv_sb[:, kd, :],
                                     start=(kd == 0), stop=(kd == 15))
                nc.vector.tensor_copy(v_sbs[T][:, i, :], ps[:])

        def proj_phase(T, x_pre=None):
            proj_v(T, proj_qk(T, x_pre))

        def _chunk(kb, h, q_sb, ps_zt, den, qlo, qhi, mask_idx,
                   z_start, z_stop, den_first):
            """One 128-key score/exp/den/z step over queries [qlo, qhi)."""
            w = qhi - qlo
            ps = ps_s.tile([P, 512], f32, tag="s")
            nc.tensor.matmul(
                ps[:, :w],
                lhsT=k_sbs[kb // 4][:, h, P * (kb % 4):P * (kb % 4 + 1)],
                rhs=q_sb[:, h, qlo:qhi],
                start=True, stop=True, skip_group_check=True)
            et = ep.tile([P, 512], f16, tag="et")
            nc.scalar.activation(et[:, :w], ps[:, :w], Exp, bias=ebias_sb[:])
            if mask_idx is not None:
                c = 384 - 128 * mask_idx
                em = emp.tile([P, 512], f16, tag="em")
                nc.vector.tensor_mul(em[:, :w], et[:, :w],
                                     masks_sb[:, c:c + w])
                e_use = em
            else:
                e_use = et
            if den_first:
                nc.vector.tensor_copy(den[:, qlo:qhi], e_use[:, :w])
            else:
                nc.vector.tensor_add(den[:, qlo:qhi], den[:, qlo:qhi],
                                     e_use[:, :w])
            nc.tensor.matmul(
                ps_zt[:, qlo:qhi],
                lhsT=v_sbs[kb // 4][:, kb % 4, P * h:P * (h + 1)],
                rhs=e_use[:, :w],
                start=z_start, stop=z_stop, skip_group_check=True)

        def attn_phase(T):
            """Head-major attention for phases 0..NP-2: shared 512-wide
            rectangle + 256-wide diagonal sub-blocks."""
            q_sb = q_sbs.pop(T)
            z_sb = zp.tile([P, HPC, 512], f16, tag="z", name=f"z_sb{T}")
            for h in range(HPC):
                ps_zt = ps_z.tile([P, 512], f32, tag="z")
                den = dp.tile([P, 512], f16, tag="den")
                for kb in range(4 * T):  # full-width rectangle
                    _chunk(kb, h, q_sb, ps_zt, den, 0, 512, None,
                           z_start=(kb == 0), z_stop=False,
                           den_first=(kb == 0))
                for i in range(2):       # 256-wide diagonal
                    for j in range(2 * (i + 1)):
                        _chunk(4 * T + j, h, q_sb, ps_zt, den,
                               256 * i, 256 * (i + 1),
                               (j - 2 * i) if j >= 2 * i else None,
                               z_start=(T == 0 and j == 0),
                               z_stop=(j == 2 * i + 1),
                               den_first=(T == 0 and j == 0))
                ds = bp.tile([P, 512], f32, tag="ds")
                nc.gpsimd.partition_all_reduce(ds[:], den[:], channels=P,
                                               reduce_op=bass_isa.ReduceOp.add)
                bc = bp.tile([P, 512], f32, tag="bc")
                nc.vector.reciprocal(bc[:], ds[:])
                nc.vector.tensor_mul(z_sb[:, h, :], ps_zt[:], bc[:])
            z_sbs[T] = z_sb

        def attn3_rect(T):
            """Last phase, stage 1: full-width rectangle (keys < 512T) for
            all heads.  Emitted between proj_qk(T) and proj_v(T) so its
            exp load runs under the projection instead of in the tail."""
            q_sb = q_sbs[T]
            zts, dens = [], []
            for h in range(HPC):
                ps_zt = ps_z.tile([P, 512], f32, tag="z", name=f"z3r{h}")
                den = dp.tile([P, 512], f16, tag="den", name=f"den3{h}")
                for kb in range(4 * T):
                    _chunk(kb, h, q_sb, ps_zt, den, 0, 512, None,
                           z_start=(kb == 0), z_stop=False,
                           den_first=(kb == 0))
                zts.append(ps_zt)
                dens.append(den)
            return zts, dens

        def attn3_diag(T, lo, hi, chunks, zts, dens, z_sb):
            """Last phase, stage 2: diagonal chunks for queries [lo, hi),
            then normalize that slice of z (feeds wo_last_part)."""
            q_sb = q_sbs[T]
            for h in range(HPC):
                for n, (kb, mi) in enumerate(chunks):
                    _chunk(kb, h, q_sb, zts[h], dens[h], lo, hi, mi,
                           z_start=False, z_stop=(n == len(chunks) - 1),
                           den_first=False)
                ds = bp.tile([P, 512], f32, tag="ds")
                nc.gpsimd.partition_all_reduce(
                    ds[:, lo:hi], dens[h][:, lo:hi], channels=P,
                    reduce_op=bass_isa.ReduceOp.add)
                bc = bp.tile([P, 512], f32, tag="bc")
                nc.vector.reciprocal(bc[:, lo:hi], ds[:, lo:hi])
                nc.vector.tensor_mul(z_sb[:, h, lo:hi],
                                     zts[h][:, lo:hi], bc[:, lo:hi])

        def wo_phase(T):
            z_sb = z_sbs.pop(T)
            rs_r = rs_in[T].rearrange("(g mi p) n -> p g mi n", p=P, mi=4)
            for g in range(4):
                o4 = op_.tile([P, 4, 512], f16, tag="o_t")
                for mi in range(4):
                    m = 4 * g + mi
                    ps = ps_z.tile([P, 512], f32, tag="z", name=f"pso{T}{m}")
                    for kd in range(HPC):
                        nc.tensor.matmul(ps[:],
                                         lhsT=wo_sb[:, kd, P * m:P * (m + 1)],
                                         rhs=z_sb[:, kd, :],
                                         start=(kd == 0), stop=(kd == HPC - 1))
                    if m % 2 == 0:
                        nc.scalar.copy(o4[:, mi, :], ps[:])
                    else:
                        nc.vector.tensor_copy(o4[:, mi, :], ps[:])
                nc.gpsimd.dma_start(out=rs_r[:, g, :, :], in_=o4[:])
            nc.gpsimd.collective_compute(
                "ReduceScatter", mybir.AluOpType.add, replica_groups=GROUPS,
                ins=[rs_in[T][:, :]], outs=[rs_out[T][:, :]])
            nc.sync.dma_start(out=out_sh[T, :, :], in_=rs_out[T][:, :])

        def wo_last_part(u, lo, hi, z_sb):
            w = hi - lo
            # rs writes go out on an idle HWDGE queue (SP for the first
            # part, ACT for the second) -- Pool's SWDGE path serializes
            # ~1.1us per descriptor-gen right before the tail collectives
            dma_eng = nc.sync if u == 0 else nc.scalar
            rs_r = rs_in_h[u].rearrange("(g mi p) n -> p g mi n", p=P, mi=4)
            for g in range(4):
                o4 = op_.tile([P, 4, 512], f16, tag="o_t")
                for mi in range(4):
                    m = 4 * g + mi
                    ps = ps_z.tile([P, 512], f32, tag="z", name=f"psoh{u}{m}")
                    for kd in range(HPC):
                        nc.tensor.matmul(
                            ps[:, :w],
                            lhsT=wo_sb[:, kd, P * m:P * (m + 1)],
                            rhs=z_sb[:, kd, lo:hi],
                            start=(kd == 0), stop=(kd == HPC - 1),
                            skip_group_check=True)
                    nc.vector.tensor_copy(o4[:, mi, 0:w], ps[:, :w])
                dma_eng.dma_start(out=rs_r[:, g, :, :], in_=o4[:, :, 0:w])
            nc.gpsimd.collective_compute(
                "ReduceScatter", mybir.AluOpType.add, replica_groups=GROUPS,
                ins=[rs_in_h[u][:, :]], outs=[rs_out_h[u][:, :]])
            dma_eng.dma_start(out=out_sh[NP - 1, :, lo:hi],
                              in_=rs_out_h[u][:, :])

        TL = NP - 1
        for T in range(TL):
            if T >= 1:
                attn_phase(T - 1)
                wo_phase(T - 1)
            proj_phase(T, x_pre=x_sb0 if T == 0 else None)
        attn_phase(TL - 1)
        x3 = proj_qk(TL)
        wo_phase(TL - 1)
        z_last = zp.tile([P, HPC, 512], f16, tag="z", name="z_last")
        zts, dens = attn3_rect(TL)
        proj_v(TL, x3, chunks=(0,))
        attn3_diag(TL, 0, 128, [(4 * TL, 0)], zts, dens, z_last)
        wo_last_part(0, 0, 128, z_last)
        proj_v(TL, x3, chunks=(1, 2, 3))
        attn3_diag(TL, 128, 512,
                   [(4 * TL, None), (4 * TL + 1, 0),
                    (4 * TL + 2, 1), (4 * TL + 3, 2)],
                   zts, dens, z_last)
        wo_last_part(1, 128, 512, z_last)
        q_sbs.pop(TL)

    nc.compile()
    return nc


_BUILT = {}


def _get_built(S):
    if S not in _BUILT:
        _BUILT[S] = _build(S)
    return _BUILT[S]


def host_inputs(x, w_qkv, w_o):
    """Build the 8 per-core input maps from full inputs."""
    B, S, D_ = x.shape
    scale = np.float32(DH) ** -0.5

    j = np.arange(0, DH, 2, dtype=np.float32) / DH
    inv_freq = (1.0 / (ROPE_BASE ** j)).astype(np.float32)
    t = np.arange(S, dtype=np.float32)
    freqs = np.outer(inv_freq, t)                            # [64, S]
    emb = np.concatenate([freqs, freqs], axis=0)             # [128, S]
    cos_t = np.cos(emb)
    sin_t = np.sin(emb)
    cosq_t = (cos_t * scale).astype(np.float16)
    sinq_t = (sin_t * scale).astype(np.float16)
    cosk_t = cos_t.astype(np.float16)
    sink_t = sin_t.astype(np.float16)

    # masks[k, u] = (u >= k + 384): slicing at [384+c : 384+c+w] yields the
    # causal mask (q >= k + c) for a 128-key chunk against w queries
    u_idx = np.arange(1024)[None, :]
    k_idx = np.arange(P)[:, None]
    masks_np = (u_idx >= k_idx + 384).astype(np.float16)     # [128, 1024]

    wqkvT = w_qkv.T.astype(np.float16)       # [D, 3D]
    woT_full = w_o.T.astype(np.float16)      # [D(in), D(out)]
    xTb = [np.ascontiguousarray(x[b].T).astype(np.float16) for b in range(2)]

    in_maps = []
    for c in range(8):
        b, r = c // 4, c % 4
        in_maps.append({
            "xT": xTb[b],
            "wqT": np.ascontiguousarray(wqkvT[:, 512 * r:512 * (r + 1)]),
            "wkT": np.ascontiguousarray(
                wqkvT[:, D + 512 * r:D + 512 * (r + 1)]),
            "wvT": np.ascontiguousarray(
                wqkvT[:, 2 * D + 512 * r:2 * D + 512 * (r + 1)]),
            "woT": np.ascontiguousarray(woT_full[512 * r:512 * (r + 1), :]),
            "cosq": cosq_t, "sinq": sinq_t,
            "cosk": cosk_t, "sink": sink_t,
            "masks": masks_np,
        })
    return in_maps


def assemble(results, B, S):
    NP = S // 512
    out = np.empty((B, S, D), dtype=np.float32)
    for c in range(8):
        b, r = c // 4, c % 4
        sh = results[c]["out_sh"]  # [NP, 512(dout), 512(tok)] fp16
        for T in range(NP):
            out[b, 512 * T:512 * (T + 1), 512 * r:512 * (r + 1)] = \
                sh[T].T.astype(np.float32)
    return out


def kernel(x, w_qkv, w_o, _trace=False):
    x = np.asarray(x, dtype=np.float32)
    w_qkv = np.asarray(w_qkv, dtype=np.float32)
    w_o = np.asarray(w_o, dtype=np.float32)
    B, S, _ = x.shape
    nc = _get_built(S)
    in_maps = host_inputs(x, w_qkv, w_o)

    def _run():
        try:
            return run_bass_kernel_spmd(nc, in_maps, list(range(8)),
                                        trace=_trace)
        except ModuleNotFoundError:
            return run_bass_kernel_spmd(nc, in_maps, list(range(8)))

    try:
        res = _run()
    except Exception:
        res = _run()  # transient runtime/readback errors: retry once
    out = assemble(res.results, B, S)
    if _trace:
        return out, res
    return out
